# revision 10
# baseline (speedup 1.0000x reference)
"""Trainium2 Bass kernel for nn_Decoder: dense MLP (sigmoid) + fixed-COO sparse matmul.

Computation:
    h = sigmoid(w @ W1.T + b1)                       # [B=128, H=8192]
    out_sp[b, r] = sum_e{rows[e]==r} edge_vals[e] * h[b, cols[e]]   # [B, OUT=32768]
    out = scale * out_sp + ref

Strategy (8 NeuronCores, SPMD):
  - Row-partitioned SpMM: core k owns output rows [4096k, 4096(k+1)).
  - Dense stage replicated on every core: h_T ([H,128] bf16 rows) computed on
    PE + ACT and staged to a DRAM scratch.
  - Sparse stage: ELL format (W nnz/row).  For each block of 128 output rows,
    dma_gather pulls the 128*W needed h_T rows (256B each) from DRAM into
    SBUF in edge-slot order; PE contracts each gathered K=128 chunk against a
    tiny host-prepared scatter weight tile (one nonzero per K-row) producing
    psum[b, r] slices directly.  scale/ref applied on DVE during psum
    evacuation.

Host-side staging is pure layout work (transpose/permute/pad/scatter of
input values, int16 index tables); all arithmetic runs on device.
"""

import numpy as np

import concourse.bass as bass
import concourse.mybir as mybir
from concourse.tile import TileContext
from concourse.bass_utils import run_bass_kernel_spmd

LATENT, HIDDEN, OUT, BATCH = 256, 8192, 32768, 128
NCORES = 8
RPC = OUT // NCORES          # rows per core = 4096
BLK = 128                    # output rows per block
NBLK = RPC // BLK            # 32 blocks per core
GB = 2                       # blocks per gather group

_NC_CACHE = {}


def _drain_barrier(tc, nc):
    """strict_bb_all_engine_barrier, but with an SP drain instead of a nop.

    The CTRL nop ISA struct only fits a couple of embedded sync waits; a
    barrier absorbing many DMA-lane completions overflows it and walrus
    errors with "Too many sync wait commands".  InstDrain lowers its waits
    through the drain mechanism and takes arbitrarily many.
    """
    from concourse.tile_rust import add_dep_helper

    curr_bb = nc.cur_bb
    assert curr_bb is not None
    prev = list(curr_bb.bb.instructions)
    b = nc.sync.drain(fusable=False)
    tc.barrier_instruction_and_bb = (b.ins, curr_bb)
    if (
        tc.no_sync_barrier_and_bb is not None
        and tc.no_sync_barrier_and_bb[1] == curr_bb
    ):
        tc.no_sync_barrier_and_bb = None
    for inst in prev:
        add_dep_helper(
            b.ins,
            inst,
            sync=bass.sync_unless_reorderable_target(inst, inst.is_executable()),
            reason="drain_barrier: backward edge",
        )


def _split_multiwaits(nc):
    """walrus codegen embeds at most ONE sync wait per ISA instruction and
    errors with "Too many sync wait commands" otherwise.  Split extra waits
    into single-wait NoOps on the same engine immediately before the
    instruction (engine streams keep program order through walrus)."""
    for f in nc.m.functions:
        for bb in f.blocks:
            out, changed = [], False
            for ins in bb.instructions:
                si = ins.sync_info
                waits = list(si.on_wait) if si and si.on_wait else []
                if len(waits) > 1:
                    changed = True
                    for wsub in waits[:-1]:
                        n = mybir.InstNoOp(name=f"I-{nc.next_id()}", ins=[], outs=[])
                        n.engine = ins.engine
                        n.sync_info = mybir.SyncInfo(on_wait=[wsub], on_update=[])
                        out.append(n)
                    ins.sync_info = mybir.SyncInfo(
                        on_wait=waits[-1:], on_update=list(si.on_update or [])
                    )
                out.append(ins)
            if changed:
                bb.instructions = out


def _build_nc(W: int):
    """Build the SPMD Bass module. W = ELL width (nnz slots per output row)."""
    R = 128 // W             # output rows covered per 128-edge K-chunk
    fp32 = mybir.dt.float32
    bf16 = mybir.dt.bfloat16
    i16 = mybir.dt.int16
    SIG = mybir.ActivationFunctionType.Sigmoid

    nc = bass.Bass("TRN2", target_bir_lowering=False, debug=False)

    d_w1t = nc.dram_tensor("w1t", [LATENT, HIDDEN], fp32, kind="ExternalInput")
    d_wt = nc.dram_tensor("wt", [LATENT, BATCH], fp32, kind="ExternalInput")
    d_b1c = nc.dram_tensor("b1c", [128, HIDDEN // 128], fp32, kind="ExternalInput")
    d_idx = nc.dram_tensor("idx", [128, NBLK * 8 * W], i16, kind="ExternalInput")
    d_enar = nc.dram_tensor("enar", [128, NBLK * W, R], fp32, kind="ExternalInput")
    d_sc = nc.dram_tensor("screp", [128, RPC], fp32, kind="ExternalInput")
    d_rf = nc.dram_tensor("refrep", [128, RPC], fp32, kind="ExternalInput")
    d_out = nc.dram_tensor("out", [BATCH, RPC], fp32, kind="ExternalOutput")

    HB = HIDDEN // 128       # 64 hidden blocks

    with TileContext(nc) as tc:
        with (
            tc.tile_pool(name="consts", bufs=1) as consts,
            tc.tile_pool(name="dram", bufs=1, space="DRAM") as drams,
            tc.tile_pool(name="work", bufs=2) as work,
            tc.tile_pool(name="psA", bufs=2, space="PSUM") as psA,
            tc.tile_pool(name="psB", bufs=2, space="PSUM") as psB,
            tc.tile_pool(name="gath", bufs=2) as gath,
        ):
            # ---------------- constant loads ----------------
            sb_w1t = consts.tile([128, 2, HIDDEN], bf16)
            nc.gpsimd.dma_start(
                out=sb_w1t[:],
                in_=d_w1t.ap().rearrange("(kc p) h -> p kc h", p=128),
            )
            sb_wt = consts.tile([128, 2, BATCH], bf16)
            nc.gpsimd.dma_start(
                out=sb_wt[:],
                in_=d_wt.ap().rearrange("(kc p) b -> p kc b", p=128),
            )
            sb_b1 = consts.tile([128, HB], fp32)
            nc.sync.dma_start(out=sb_b1[:], in_=d_b1c.ap())
            sb_idx = consts.tile([128, NBLK * 8 * W], i16)
            nc.sync.dma_start(out=sb_idx[:], in_=d_idx.ap())
            sb_enar = consts.tile([128, NBLK * W, R], bf16)
            nc.gpsimd.dma_start(out=sb_enar[:], in_=d_enar.ap())
            sb_sc = consts.tile([128, RPC], fp32)
            nc.sync.dma_start(out=sb_sc[:], in_=d_sc.ap())
            sb_rf = consts.tile([128, RPC], fp32)
            nc.sync.dma_start(out=sb_rf[:], in_=d_rf.ap())

            # All const loads land before compute: collapses the many per-lane
            # DMA waits (ACT's ISA struct has a single sync-wait slot).
            _drain_barrier(tc, nc)

            # ---------------- stage A: h_T -> DRAM ----------------
            ht_dram = drams.tile([HIDDEN, BATCH], bf16)
            ht_sb = consts.tile([128, HB, BATCH], bf16)
            for quad in range(HB // 4):
                ps = psA.tile([128, 512], fp32, tag="hps")
                for i4 in range(4):
                    i = quad * 4 + i4
                    for k in range(2):
                        nc.tensor.matmul(
                            ps[:, i4 * 128 : (i4 + 1) * 128],
                            lhsT=sb_w1t[:, k, i * 128 : (i + 1) * 128],
                            rhs=sb_wt[:, k, :],
                            start=(k == 0),
                            stop=(k == 1),
                        )
                    nc.scalar.activation(
                        ht_sb[:, i, :],
                        ps[:, i4 * 128 : (i4 + 1) * 128],
                        SIG,
                        bias=sb_b1[:, i : i + 1],
                        scale=1.0,
                    )
            nc.sync.dma_start(
                out=ht_dram[:].rearrange("(i p) b -> p i b", p=128),
                in_=ht_sb[:],
            )
            # Gathers read ht_dram; absorb the store-DMA wait here.
            _drain_barrier(tc, nc)

            # ---------------- stage B: gather + sparse matmul ----------------
            # dma_gather is an extended GpSimd instruction: load the Q7
            # library holding it (Bacc does this automatically; raw Bass
            # doesn't).  no_sync deps pin it before every gather.
            from concourse import library_config
            from concourse.tile_rust import add_dep_helper

            libload = nc.gpsimd.load_library(library_config.mlp)

            ht_src = ht_dram[:]          # [HIDDEN, BATCH] AP, 256B rows
            ps_out = None
            for grp in range(NBLK // GB):
                gt = gath.tile([128, GB * W, BATCH], bf16, tag="g")
                ni = 128 * W * GB
                g_inst = nc.gpsimd.dma_gather(
                    gt[:],
                    ht_src,
                    sb_idx[:, grp * 8 * W * GB : (grp + 1) * 8 * W * GB],
                    num_idxs=ni,
                    num_idxs_reg=ni,
                    elem_size=BATCH,
                    # >~1K idxs in one packet hard-crashes the SDMA
                    # (NRT_EXEC_UNIT_UNRECOVERABLE); let SWDGE packetize.
                    single_packet=False,
                )
                add_dep_helper(
                    g_inst.ins, libload.ins, sync=False, reason="lib before gather"
                )
                for b2 in range(GB):
                    rb = grp * GB + b2
                    if rb % 4 == 0:
                        ps_out = psB.tile([128, 512], fp32, tag="ops")
                    base = (rb % 4) * 128
                    for g in range(W):
                        nc.tensor.matmul(
                            ps_out[:, base + g * R : base + (g + 1) * R],
                            lhsT=gt[:, b2 * W + g, :],
                            rhs=sb_enar[:, rb * W + g, :],
                            start=True,
                            stop=True,
                        )
                    if rb % 4 == 3:
                        q = rb // 4
                        tmp = work.tile([128, 512], fp32, tag="tmp")
                        nc.vector.tensor_mul(
                            out=tmp[:],
                            in0=ps_out[:],
                            in1=sb_sc[:, q * 512 : (q + 1) * 512],
                        )
                        ot = work.tile([128, 512], fp32, tag="ot")
                        nc.vector.tensor_add(
                            out=ot[:],
                            in0=tmp[:],
                            in1=sb_rf[:, q * 512 : (q + 1) * 512],
                        )
                        nc.sync.dma_start(
                            out=d_out.ap()[:, q * 512 : (q + 1) * 512], in_=ot[:]
                        )
    from concourse.library_overlay import lower_extended_insts

    lower_extended_insts(nc)  # populate .instr for InstISA subclasses
    _split_multiwaits(nc)
    return nc


def _stage_inputs(w, W1, b1, edge_vals, rows, cols, scale, ref):
    """Pure-layout host staging -> list of per-core input dicts + ELL width."""
    f32 = np.float32
    w = np.asarray(w, dtype=f32)
    W1 = np.asarray(W1, dtype=f32)
    b1 = np.asarray(b1, dtype=f32)
    edge_vals = np.asarray(edge_vals, dtype=f32)
    rows = np.asarray(rows, dtype=np.int64)
    cols = np.asarray(cols, dtype=np.int64)
    scale = np.asarray(scale, dtype=f32)
    ref = np.asarray(ref, dtype=f32)

    nnz = rows.shape[0]
    # Ensure row-sorted edge order (stable keeps reference's intra-row order).
    if np.any(rows[1:] < rows[:-1]):
        order = np.argsort(rows, kind="stable")
        rows, cols, edge_vals = rows[order], cols[order], edge_vals[order]

    counts = np.bincount(rows, minlength=OUT)
    maxc = int(counts.max())
    W = 32 if maxc <= 32 else (64 if maxc <= 64 else 128)
    assert maxc <= 128, f"row with {maxc} nnz unsupported"
    R = 128 // W

    # ELL (padded) arrays [OUT, W]; pads use col 0 with value 0.
    off = np.zeros(OUT + 1, dtype=np.int64)
    np.cumsum(counts, out=off[1:])
    pos = np.arange(nnz, dtype=np.int64) - off[rows]
    ell_col = np.zeros((OUT, W), dtype=np.int16)
    ell_val = np.zeros((OUT, W), dtype=f32)
    ell_col[rows, pos] = cols.astype(np.int16)
    ell_val[rows, pos] = edge_vals

    w1t = np.ascontiguousarray(W1.T)                      # [LATENT, HIDDEN]
    wt = np.ascontiguousarray(w.T)                        # [LATENT, BATCH]
    b1c = np.ascontiguousarray(b1.reshape(HIDDEN // 128, 128).T)  # [128, 64]

    parts = np.arange(128)
    in_maps = []
    for k in range(NCORES):
        rs = slice(k * RPC, (k + 1) * RPC)
        cvals = ell_val[rs]                               # [RPC, W]
        ccols = ell_col[rs]                               # [RPC, W]

        # edge-slot order inside a 128-row block: slot = r_in_block*W + j
        slot_col = ccols.reshape(NBLK, 128 * W)           # [NBLK, 128*W]
        slot_val = cvals.reshape(NBLK, 128 * W)

        # idx table: idx[p, rb*8W + s] = slot_col[rb, s*16 + p%16]
        a = slot_col.reshape(NBLK, 8 * W, 16)             # [NBLK, s, t]
        a = a.transpose(2, 0, 1).reshape(16, NBLK * 8 * W)
        idx = np.ascontiguousarray(np.tile(a, (8, 1)))    # [128, NBLK*8W]

        # E_nar: enar[p, rb*W + g, c] = slot_val[rb, 128g+p] iff c == p//W
        vgp = slot_val.reshape(NBLK, W, 128)              # [rb, g, p]
        enar = np.zeros((128, NBLK * W, R), dtype=f32)
        enar[parts[:, None], np.arange(NBLK * W)[None, :], (parts // W)[:, None]] = (
            vgp.transpose(2, 0, 1).reshape(128, NBLK * W)
        )

        screp = np.ascontiguousarray(
            np.broadcast_to(scale[rs][None, :], (128, RPC))
        )
        refrep = np.ascontiguousarray(
            np.broadcast_to(ref[rs][None, :], (128, RPC))
        )
        in_maps.append(
            {
                "w1t": w1t,
                "wt": wt,
                "b1c": b1c,
                "idx": idx,
                "enar": np.ascontiguousarray(enar),
                "screp": screp,
                "refrep": refrep,
            }
        )
    return in_maps, W


def kernel(w, W1, b1, edge_vals, rows, cols, scale, ref):
    in_maps, W = _stage_inputs(w, W1, b1, edge_vals, rows, cols, scale, ref)
    if W not in _NC_CACHE:
        _NC_CACHE[W] = _build_nc(W)
    nc = _NC_CACHE[W]
    res = run_bass_kernel_spmd(nc, in_maps, core_ids=list(range(NCORES)))
    out = np.concatenate([r["out"] for r in res.results], axis=1)
    return out.astype(np.float32)


if __name__ == "__main__":
    rng = np.random.default_rng(0)
    nnz = OUT * 32
    ins = {
        "w": rng.standard_normal((BATCH, LATENT), dtype=np.float32),
        "W1": rng.standard_normal((HIDDEN, LATENT), dtype=np.float32),
        "b1": rng.standard_normal(HIDDEN, dtype=np.float32) * 0.01,
        "edge_vals": rng.standard_normal(nnz, dtype=np.float32),
        "rows": np.repeat(np.arange(OUT, dtype=np.int64), 32),
        "cols": rng.integers(0, HIDDEN, nnz).astype(np.int64),
        "scale": rng.random(OUT, dtype=np.float32) + 0.5,
        "ref": rng.standard_normal(OUT, dtype=np.float32),
    }
    out = kernel(**ins)
    print(out.shape, out.dtype)


# revision 11
# speedup vs baseline: 4.5165x; 4.5165x over previous
"""Trainium2 Bass kernel for nn_Decoder: dense MLP (sigmoid) + fixed-COO sparse matmul.

Computation:
    h = sigmoid(w @ W1.T + b1)                       # [B=128, H=8192]
    out_sp[b, r] = sum_e{rows[e]==r} edge_vals[e] * h[b, cols[e]]   # [B, OUT=32768]
    out = scale * out_sp + ref

Strategy (8 NeuronCores, SPMD, row-partitioned):
  - Core k owns output rows [4096k, 4096(k+1)).
  - The sparse matrix is canonicalized host-side (COO -> dense per-core
    slice, duplicates summed, bf16) and streamed tile-by-tile from HBM:
    measured dma_gather descriptor generation costs ~8 ns/edge on the Q7
    (1 ms total), while streaming the 64 MB/core dense slice runs at full
    HBM rate (~180 us) and turns the whole sparse stage into back-to-back
    dense matmuls with h resident in SBUF.
  - Dense stage replicated on every core: h_T kept in SBUF as the matmul
    lhsT; bias+sigmoid fused on ACT; W1 transposed host-side (layout only)
    and cast to bf16 during the SWDGE load.
  - scale/ref fused into the PSUM evacuation on DVE.
"""

import numpy as np
import ml_dtypes

import concourse.bass as bass
import concourse.mybir as mybir
from concourse.tile import TileContext
from concourse.bass_utils import run_bass_kernel_spmd

LATENT, HIDDEN, OUT, BATCH = 256, 8192, 32768, 128
NCORES = 8
RPC = OUT // NCORES          # rows per core = 4096
RBLK = 512                   # output rows per matmul block (PSUM bank)
NRB = RPC // RBLK            # 8 row blocks per core
HB = HIDDEN // 128           # 64 hidden chunks

_NC_CACHE = {}


def _drain_barrier(tc, nc):
    """strict_bb_all_engine_barrier, but with an SP drain instead of a nop
    (the nop ISA struct can't hold the many DMA-lane waits)."""
    from concourse.tile_rust import add_dep_helper

    curr_bb = nc.cur_bb
    assert curr_bb is not None
    prev = list(curr_bb.bb.instructions)
    b = nc.sync.drain(fusable=False)
    tc.barrier_instruction_and_bb = (b.ins, curr_bb)
    if (
        tc.no_sync_barrier_and_bb is not None
        and tc.no_sync_barrier_and_bb[1] == curr_bb
    ):
        tc.no_sync_barrier_and_bb = None
    for inst in prev:
        add_dep_helper(
            b.ins,
            inst,
            sync=bass.sync_unless_reorderable_target(inst, inst.is_executable()),
            reason="drain_barrier: backward edge",
        )


def _split_multiwaits(nc):
    """walrus codegen embeds at most ONE sync wait per ISA instruction and
    errors with "Too many sync wait commands" otherwise.  Split extra waits
    into single-wait NoOps on the same engine immediately before the
    instruction (engine streams keep program order through walrus)."""
    for f in nc.m.functions:
        for bb in f.blocks:
            out, changed = [], False
            for ins in bb.instructions:
                si = ins.sync_info
                waits = list(si.on_wait) if si and si.on_wait else []
                if len(waits) > 1:
                    changed = True
                    for wsub in waits[:-1]:
                        n = mybir.InstNoOp(name=f"I-{nc.next_id()}", ins=[], outs=[])
                        n.engine = ins.engine
                        n.sync_info = mybir.SyncInfo(on_wait=[wsub], on_update=[])
                        out.append(n)
                    ins.sync_info = mybir.SyncInfo(
                        on_wait=waits[-1:], on_update=list(si.on_update or [])
                    )
                out.append(ins)
            if changed:
                bb.instructions = out


def _build_nc():
    fp32 = mybir.dt.float32
    bf16 = mybir.dt.bfloat16
    SIG = mybir.ActivationFunctionType.Sigmoid

    nc = bass.Bass("TRN2", target_bir_lowering=False, debug=False)

    d_w1t = nc.dram_tensor("w1t", [LATENT, HIDDEN], fp32, kind="ExternalInput")
    d_wt = nc.dram_tensor("wt", [LATENT, BATCH], fp32, kind="ExternalInput")
    d_b1c = nc.dram_tensor("b1c", [128, HB], fp32, kind="ExternalInput")
    d_s = nc.dram_tensor("sdense", [NRB, HB * 128, RBLK], bf16, kind="ExternalInput")
    d_sc = nc.dram_tensor("screp", [128, RPC], fp32, kind="ExternalInput")
    d_rf = nc.dram_tensor("refrep", [128, RPC], fp32, kind="ExternalInput")
    d_out = nc.dram_tensor("out", [BATCH, RPC], fp32, kind="ExternalOutput")

    with TileContext(nc) as tc:
        with (
            tc.tile_pool(name="consts", bufs=1) as consts,
            tc.tile_pool(name="wstream", bufs=2) as wstream,
            tc.tile_pool(name="sstream", bufs=2) as sstream,
            tc.tile_pool(name="work", bufs=2) as work,
            tc.tile_pool(name="psA", bufs=2, space="PSUM") as psA,
            tc.tile_pool(name="psB", bufs=2, space="PSUM") as psB,
        ):
            # ---------------- constant loads ----------------
            sb_wt = consts.tile([128, 2, BATCH], bf16)
            nc.gpsimd.dma_start(
                out=sb_wt[:],
                in_=d_wt.ap().rearrange("(kc p) b -> p kc b", p=128),
            )
            sb_b1 = consts.tile([128, HB], fp32)
            nc.sync.dma_start(out=sb_b1[:], in_=d_b1c.ap())
            sb_sc = consts.tile([128, RPC], fp32)
            nc.sync.dma_start(out=sb_sc[:], in_=d_sc.ap())
            sb_rf = consts.tile([128, RPC], fp32)
            nc.sync.dma_start(out=sb_rf[:], in_=d_rf.ap())

            # Collapse the per-lane DMA waits (most ISA structs hold 1 wait).
            _drain_barrier(tc, nc)

            # ---------------- stage A: h_T stays in SBUF ----------------
            ht_sb = consts.tile([128, HB, BATCH], bf16)
            w1t_ap = d_w1t.ap().rearrange("(kc p) h -> p kc h", p=128)
            for quad in range(HB // 4):
                wq = wstream.tile([128, 2, 512], bf16, tag="wq")
                nc.gpsimd.dma_start(
                    out=wq[:], in_=w1t_ap[:, :, quad * 512 : (quad + 1) * 512]
                )
                ps = psA.tile([128, 512], fp32, tag="hps")
                for i4 in range(4):
                    i = quad * 4 + i4
                    for k in range(2):
                        nc.tensor.matmul(
                            ps[:, i4 * 128 : (i4 + 1) * 128],
                            lhsT=wq[:, k, i4 * 128 : (i4 + 1) * 128],
                            rhs=sb_wt[:, k, :],
                            start=(k == 0),
                            stop=(k == 1),
                        )
                    nc.scalar.activation(
                        ht_sb[:, i, :],
                        ps[:, i4 * 128 : (i4 + 1) * 128],
                        SIG,
                        bias=sb_b1[:, i : i + 1],
                        scale=1.0,
                    )

            # ---------------- stage B: dense S matmul ----------------
            for rb in range(NRB):
                st = sstream.tile([128, HB, RBLK], bf16, tag="s")
                nc.sync.dma_start(
                    out=st[:],
                    in_=d_s.ap()[rb].rearrange("(cc p) r -> p cc r", p=128),
                )
                ps = psB.tile([128, RBLK], fp32, tag="ops")
                for cc in range(HB):
                    nc.tensor.matmul(
                        ps[:],
                        lhsT=ht_sb[:, cc, :],
                        rhs=st[:, cc, :],
                        start=(cc == 0),
                        stop=(cc == HB - 1),
                    )
                tmp = work.tile([128, RBLK], fp32, tag="tmp")
                nc.vector.tensor_mul(
                    out=tmp[:], in0=ps[:], in1=sb_sc[:, rb * RBLK : (rb + 1) * RBLK]
                )
                ot = work.tile([128, RBLK], fp32, tag="ot")
                nc.vector.tensor_add(
                    out=ot[:], in0=tmp[:], in1=sb_rf[:, rb * RBLK : (rb + 1) * RBLK]
                )
                nc.sync.dma_start(
                    out=d_out.ap()[:, rb * RBLK : (rb + 1) * RBLK], in_=ot[:]
                )

    _split_multiwaits(nc)
    return nc


def _stage_inputs(w, W1, b1, edge_vals, rows, cols, scale, ref):
    """Pure-layout host staging: transposes, COO->dense canonicalization
    (duplicates summed, scipy-style), bf16 packing. No model arithmetic."""
    f32 = np.float32
    w = np.asarray(w, dtype=f32)
    W1 = np.asarray(W1, dtype=f32)
    b1 = np.asarray(b1, dtype=f32)
    edge_vals = np.asarray(edge_vals, dtype=f32)
    rows = np.asarray(rows, dtype=np.int64)
    cols = np.asarray(cols, dtype=np.int64)
    scale = np.asarray(scale, dtype=f32)
    ref = np.asarray(ref, dtype=f32)

    w1t = np.ascontiguousarray(W1.T)                      # [LATENT, HIDDEN]
    wt = np.ascontiguousarray(w.T)                        # [LATENT, BATCH]
    b1c = np.ascontiguousarray(b1.reshape(HB, 128).T)     # [128, HB]

    in_maps = []
    for k in range(NCORES):
        lo, hi = k * RPC, (k + 1) * RPC
        sel = (rows >= lo) & (rows < hi)
        r_k = rows[sel] - lo
        c_k = cols[sel]
        v_k = edge_vals[sel]

        # Dense per-core slice S[c, r], duplicate (c, r) entries summed.
        sdense = np.zeros((HIDDEN, RPC), dtype=f32)
        np.add.at(sdense, (c_k, r_k), v_k)
        # [NRB, HB*128, RBLK] layout: tile (rb, cc) contiguous per partition
        sdense = sdense.reshape(HIDDEN, NRB, RBLK).transpose(1, 0, 2)
        sdense = np.ascontiguousarray(sdense.astype(ml_dtypes.bfloat16))

        screp = np.ascontiguousarray(np.broadcast_to(scale[lo:hi][None, :], (128, RPC)))
        refrep = np.ascontiguousarray(np.broadcast_to(ref[lo:hi][None, :], (128, RPC)))
        in_maps.append(
            {
                "w1t": w1t,
                "wt": wt,
                "b1c": b1c,
                "sdense": sdense,
                "screp": screp,
                "refrep": refrep,
            }
        )
    return in_maps


def kernel(w, W1, b1, edge_vals, rows, cols, scale, ref):
    in_maps = _stage_inputs(w, W1, b1, edge_vals, rows, cols, scale, ref)
    if "nc" not in _NC_CACHE:
        _NC_CACHE["nc"] = _build_nc()
    nc = _NC_CACHE["nc"]
    res = run_bass_kernel_spmd(nc, in_maps, core_ids=list(range(NCORES)))
    out = np.concatenate([r["out"] for r in res.results], axis=1)
    return out.astype(np.float32)


if __name__ == "__main__":
    rng = np.random.default_rng(0)
    nnz = OUT * 32
    ins = {
        "w": rng.standard_normal((BATCH, LATENT), dtype=np.float32),
        "W1": rng.standard_normal((HIDDEN, LATENT), dtype=np.float32),
        "b1": rng.standard_normal(HIDDEN, dtype=np.float32) * 0.01,
        "edge_vals": rng.standard_normal(nnz, dtype=np.float32),
        "rows": np.repeat(np.arange(OUT, dtype=np.int64), 32),
        "cols": rng.integers(0, HIDDEN, nnz).astype(np.int64),
        "scale": rng.random(OUT, dtype=np.float32) + 0.5,
        "ref": rng.standard_normal(OUT, dtype=np.float32),
    }
    out = kernel(**ins)
    print(out.shape, out.dtype)


# revision 14
# speedup vs baseline: 5.0206x; 1.1116x over previous
"""Trainium2 Bass kernel for nn_Decoder: dense MLP (sigmoid) + fixed-COO sparse matmul.

Computation:
    h = sigmoid(w @ W1.T + b1)                       # [B=128, H=8192]
    out_sp[b, r] = sum_e{rows[e]==r} edge_vals[e] * h[b, cols[e]]   # [B, OUT=32768]
    out = scale * out_sp + ref

Strategy (8 NeuronCores, SPMD, row-partitioned):
  - Core k owns output rows [4096k, 4096(k+1)).
  - The sparse matrix is canonicalized host-side (COO -> dense per-core
    slice, duplicates summed, bf16) and streamed tile-by-tile from HBM:
    measured dma_gather descriptor generation costs ~8 ns/edge on the Q7
    (1 ms total), while streaming the 64 MB/core dense slice runs at full
    HBM rate (~180 us) and turns the whole sparse stage into back-to-back
    dense matmuls with h resident in SBUF.
  - Dense stage replicated on every core: h_T kept in SBUF as the matmul
    lhsT; bias+sigmoid fused on ACT; W1 transposed host-side (layout only)
    and cast to bf16 during the SWDGE load.
  - scale/ref fused into the PSUM evacuation on DVE.
"""

import numpy as np
import ml_dtypes

import concourse.bass as bass
import concourse.mybir as mybir
from concourse.tile import TileContext
from concourse.bass_utils import run_bass_kernel_spmd

LATENT, HIDDEN, OUT, BATCH = 256, 8192, 32768, 128
NCORES = 8
RPC = OUT // NCORES          # rows per core = 4096
RBLK = 512                   # output rows per matmul block (PSUM bank)
NRB = RPC // RBLK            # 8 row blocks per core
HB = HIDDEN // 128           # 64 hidden chunks

_NC_CACHE = {}


def _drain_barrier(tc, nc):
    """strict_bb_all_engine_barrier, but with an SP drain instead of a nop
    (the nop ISA struct can't hold the many DMA-lane waits)."""
    from concourse.tile_rust import add_dep_helper

    curr_bb = nc.cur_bb
    assert curr_bb is not None
    prev = list(curr_bb.bb.instructions)
    b = nc.sync.drain(fusable=False)
    tc.barrier_instruction_and_bb = (b.ins, curr_bb)
    if (
        tc.no_sync_barrier_and_bb is not None
        and tc.no_sync_barrier_and_bb[1] == curr_bb
    ):
        tc.no_sync_barrier_and_bb = None
    for inst in prev:
        add_dep_helper(
            b.ins,
            inst,
            sync=bass.sync_unless_reorderable_target(inst, inst.is_executable()),
            reason="drain_barrier: backward edge",
        )


def _split_multiwaits(nc):
    """walrus codegen embeds at most ONE sync wait per ISA instruction and
    errors with "Too many sync wait commands" otherwise.  Split extra waits
    into single-wait NoOps on the same engine immediately before the
    instruction (engine streams keep program order through walrus)."""
    for f in nc.m.functions:
        for bb in f.blocks:
            out, changed = [], False
            for ins in bb.instructions:
                si = ins.sync_info
                waits = list(si.on_wait) if si and si.on_wait else []
                if len(waits) > 1:
                    changed = True
                    for wsub in waits[:-1]:
                        n = mybir.InstNoOp(name=f"I-{nc.next_id()}", ins=[], outs=[])
                        n.engine = ins.engine
                        n.sync_info = mybir.SyncInfo(on_wait=[wsub], on_update=[])
                        out.append(n)
                    ins.sync_info = mybir.SyncInfo(
                        on_wait=waits[-1:], on_update=list(si.on_update or [])
                    )
                out.append(ins)
            if changed:
                bb.instructions = out


def _build_nc():
    fp32 = mybir.dt.float32
    bf16 = mybir.dt.bfloat16
    SIG = mybir.ActivationFunctionType.Sigmoid

    nc = bass.Bass("TRN2", target_bir_lowering=False, debug=False)

    d_w1t = nc.dram_tensor("w1t", [LATENT, HIDDEN], bf16, kind="ExternalInput")
    d_wt = nc.dram_tensor("wt", [LATENT, BATCH], bf16, kind="ExternalInput")
    d_b1c = nc.dram_tensor("b1c", [128, HB], fp32, kind="ExternalInput")
    d_s = nc.dram_tensor("sdense", [HB, 128, NRB, RBLK], bf16, kind="ExternalInput")
    d_sc = nc.dram_tensor("screp", [128, RPC], fp32, kind="ExternalInput")
    d_rf = nc.dram_tensor("refrep", [128, RPC], fp32, kind="ExternalInput")
    d_out = nc.dram_tensor("out", [BATCH, RPC], fp32, kind="ExternalOutput")

    with TileContext(nc) as tc:
        with (
            tc.tile_pool(name="consts", bufs=1) as consts,
            tc.tile_pool(name="wstream", bufs=2) as wstream,
            tc.tile_pool(name="sstream", bufs=6) as sstream,
            tc.tile_pool(name="work", bufs=2) as work,
        ):
            # ---------------- constant loads ----------------
            sb_wt = consts.tile([128, 2, BATCH], bf16)
            nc.sync.dma_start(
                out=sb_wt[:],
                in_=d_wt.ap().rearrange("(kc p) b -> p kc b", p=128),
            )
            sb_b1 = consts.tile([128, HB], fp32)
            nc.sync.dma_start(out=sb_b1[:], in_=d_b1c.ap())
            sb_sc = consts.tile([128, RPC], fp32)
            nc.sync.dma_start(out=sb_sc[:], in_=d_sc.ap())
            sb_rf = consts.tile([128, RPC], fp32)
            nc.sync.dma_start(out=sb_rf[:], in_=d_rf.ap())

            # Collapse the per-lane DMA waits (most ISA structs hold 1 wait).
            _drain_barrier(tc, nc)

            # ---------------- stage A: h_T stays in SBUF ----------------
            ht_sb = consts.tile([128, HB, BATCH], bf16)
            w1t_ap = d_w1t.ap().rearrange("(kc p) h -> p kc h", p=128)
            with tc.tile_pool(name="psA", bufs=2, space="PSUM") as psA:
                for quad in range(HB // 4):
                    wq = wstream.tile([128, 2, 512], bf16, tag="wq")
                    nc.sync.dma_start(
                        out=wq[:], in_=w1t_ap[:, :, quad * 512 : (quad + 1) * 512]
                    )
                    ps = psA.tile([128, 512], fp32, tag="hps")
                    for i4 in range(4):
                        i = quad * 4 + i4
                        for k in range(2):
                            nc.tensor.matmul(
                                ps[:, i4 * 128 : (i4 + 1) * 128],
                                lhsT=wq[:, k, i4 * 128 : (i4 + 1) * 128],
                                rhs=sb_wt[:, k, :],
                                start=(k == 0),
                                stop=(k == 1),
                            )
                        nc.scalar.activation(
                            ht_sb[:, i, :],
                            ps[:, i4 * 128 : (i4 + 1) * 128],
                            SIG,
                            bias=sb_b1[:, i : i + 1],
                            scale=1.0,
                        )

            # ---------------- stage B: dense S matmul, cc-major ----------------
            # One stationary load per hidden chunk (streams 8*512 columns),
            # 8 PSUM banks accumulate the 8 row-blocks simultaneously; keeps
            # PE streaming (HAM warm) while 1MB S slabs prefetch.
            with tc.tile_pool(name="psB", bufs=1, space="PSUM") as psB:
                pss = [
                    psB.tile([128, RBLK], fp32, tag=f"ops{rb}", name=f"ps{rb}")
                    for rb in range(NRB)
                ]
                for cc in range(HB):
                    st = sstream.tile([128, NRB, RBLK], bf16, tag="s")
                    nc.sync.dma_start(out=st[:], in_=d_s.ap()[cc])
                    for rb in range(NRB):
                        nc.tensor.matmul(
                            pss[rb][:],
                            lhsT=ht_sb[:, cc, :],
                            rhs=st[:, rb, :],
                            start=(cc == 0),
                            stop=(cc == HB - 1),
                        )
                for rb in range(NRB):
                    tmp = work.tile([128, RBLK], fp32, tag="tmp")
                    nc.vector.tensor_mul(
                        out=tmp[:],
                        in0=pss[rb][:],
                        in1=sb_sc[:, rb * RBLK : (rb + 1) * RBLK],
                    )
                    ot = work.tile([128, RBLK], fp32, tag="ot")
                    nc.vector.tensor_add(
                        out=ot[:],
                        in0=tmp[:],
                        in1=sb_rf[:, rb * RBLK : (rb + 1) * RBLK],
                    )
                    nc.sync.dma_start(
                        out=d_out.ap()[:, rb * RBLK : (rb + 1) * RBLK], in_=ot[:]
                    )

    _split_multiwaits(nc)
    return nc


def _stage_inputs(w, W1, b1, edge_vals, rows, cols, scale, ref):
    """Pure-layout host staging: transposes, COO->dense canonicalization
    (duplicates summed, scipy-style), bf16 packing. No model arithmetic."""
    f32 = np.float32
    w = np.asarray(w, dtype=f32)
    W1 = np.asarray(W1, dtype=f32)
    b1 = np.asarray(b1, dtype=f32)
    edge_vals = np.asarray(edge_vals, dtype=f32)
    rows = np.asarray(rows, dtype=np.int64)
    cols = np.asarray(cols, dtype=np.int64)
    scale = np.asarray(scale, dtype=f32)
    ref = np.asarray(ref, dtype=f32)

    w1t = np.ascontiguousarray(W1.T.astype(ml_dtypes.bfloat16))  # [LATENT, HIDDEN]
    wt = np.ascontiguousarray(w.T.astype(ml_dtypes.bfloat16))    # [LATENT, BATCH]
    b1c = np.ascontiguousarray(b1.reshape(HB, 128).T)            # [128, HB]

    in_maps = []
    for k in range(NCORES):
        lo, hi = k * RPC, (k + 1) * RPC
        sel = (rows >= lo) & (rows < hi)
        r_k = rows[sel] - lo
        c_k = cols[sel]
        v_k = edge_vals[sel]

        # Dense per-core slice S[c, r], duplicate (c, r) entries summed.
        sdense = np.zeros((HIDDEN, RPC), dtype=f32)
        np.add.at(sdense, (c_k, r_k), v_k)
        # [HB, 128, NRB, RBLK]: cc-major slabs, 8KB contiguous per partition
        sdense = sdense.reshape(HB, 128, NRB, RBLK)
        sdense = np.ascontiguousarray(sdense.astype(ml_dtypes.bfloat16))

        screp = np.ascontiguousarray(np.broadcast_to(scale[lo:hi][None, :], (128, RPC)))
        refrep = np.ascontiguousarray(np.broadcast_to(ref[lo:hi][None, :], (128, RPC)))
        in_maps.append(
            {
                "w1t": w1t,
                "wt": wt,
                "b1c": b1c,
                "sdense": sdense,
                "screp": screp,
                "refrep": refrep,
            }
        )
    return in_maps


def kernel(w, W1, b1, edge_vals, rows, cols, scale, ref):
    in_maps = _stage_inputs(w, W1, b1, edge_vals, rows, cols, scale, ref)
    if "nc" not in _NC_CACHE:
        _NC_CACHE["nc"] = _build_nc()
    nc = _NC_CACHE["nc"]
    res = run_bass_kernel_spmd(nc, in_maps, core_ids=list(range(NCORES)))
    out = np.concatenate([r["out"] for r in res.results], axis=1)
    return out.astype(np.float32)


if __name__ == "__main__":
    rng = np.random.default_rng(0)
    nnz = OUT * 32
    ins = {
        "w": rng.standard_normal((BATCH, LATENT), dtype=np.float32),
        "W1": rng.standard_normal((HIDDEN, LATENT), dtype=np.float32),
        "b1": rng.standard_normal(HIDDEN, dtype=np.float32) * 0.01,
        "edge_vals": rng.standard_normal(nnz, dtype=np.float32),
        "rows": np.repeat(np.arange(OUT, dtype=np.int64), 32),
        "cols": rng.integers(0, HIDDEN, nnz).astype(np.int64),
        "scale": rng.random(OUT, dtype=np.float32) + 0.5,
        "ref": rng.standard_normal(OUT, dtype=np.float32),
    }
    out = kernel(**ins)
    print(out.shape, out.dtype)


# revision 15
# speedup vs baseline: 5.1334x; 1.0225x over previous
"""Trainium2 Bass kernel for nn_Decoder: dense MLP (sigmoid) + fixed-COO sparse matmul.

Computation:
    h = sigmoid(w @ W1.T + b1)                       # [B=128, H=8192]
    out_sp[b, r] = sum_e{rows[e]==r} edge_vals[e] * h[b, cols[e]]   # [B, OUT=32768]
    out = scale * out_sp + ref

Strategy (8 NeuronCores, SPMD, row-partitioned):
  - Core k owns output rows [4096k, 4096(k+1)).
  - The sparse matrix is canonicalized host-side (COO -> dense per-core
    slice, duplicates summed, bf16) and streamed slab-by-slab from HBM:
    measured dma_gather descriptor generation costs ~8 ns/edge on the Q7
    (1 ms total), while streaming the 64 MB/core dense slice runs at HBM
    rate (~160 us) and turns the sparse stage into dense matmuls with h
    resident in SBUF as the stationary operand (one LDWEIGHTS per hidden
    chunk, PE stays HAM-warm).
  - Dense stage replicated on every core; b1 folded in as a K=1 matmul so
    the sigmoid runs as one batched ACT per PSUM bank.
  - scale/ref fused into the PSUM evacuation on DVE; stage B runs in two
    row-half passes so the first half's evacuation overlaps the second
    half's S stream.
"""

import numpy as np
import ml_dtypes

import concourse.bass as bass
import concourse.mybir as mybir
from concourse.tile import TileContext
from concourse.bass_utils import run_bass_kernel_spmd

LATENT, HIDDEN, OUT, BATCH = 256, 8192, 32768, 128
NCORES = 8
RPC = OUT // NCORES          # rows per core = 4096
RBLK = 512                   # output rows per PSUM bank
NRB = RPC // RBLK            # 8 row blocks per core
HRB = NRB // 2               # row blocks per pass
HB = HIDDEN // 128           # 64 hidden chunks

_NC_CACHE = {}


def _drain_barrier(tc, nc):
    """strict_bb_all_engine_barrier, but with an SP drain instead of a nop
    (the nop ISA struct can't hold the many DMA-lane waits)."""
    from concourse.tile_rust import add_dep_helper

    curr_bb = nc.cur_bb
    assert curr_bb is not None
    prev = list(curr_bb.bb.instructions)
    b = nc.sync.drain(fusable=False)
    tc.barrier_instruction_and_bb = (b.ins, curr_bb)
    if (
        tc.no_sync_barrier_and_bb is not None
        and tc.no_sync_barrier_and_bb[1] == curr_bb
    ):
        tc.no_sync_barrier_and_bb = None
    for inst in prev:
        add_dep_helper(
            b.ins,
            inst,
            sync=bass.sync_unless_reorderable_target(inst, inst.is_executable()),
            reason="drain_barrier: backward edge",
        )


def _split_multiwaits(nc):
    """walrus codegen embeds at most ONE sync wait per ISA instruction and
    errors with "Too many sync wait commands" otherwise.  Split extra waits
    into single-wait NoOps on the same engine immediately before the
    instruction (engine streams keep program order through walrus)."""
    for f in nc.m.functions:
        for bb in f.blocks:
            out, changed = [], False
            for ins in bb.instructions:
                si = ins.sync_info
                waits = list(si.on_wait) if si and si.on_wait else []
                if len(waits) > 1:
                    changed = True
                    for wsub in waits[:-1]:
                        n = mybir.InstNoOp(name=f"I-{nc.next_id()}", ins=[], outs=[])
                        n.engine = ins.engine
                        n.sync_info = mybir.SyncInfo(on_wait=[wsub], on_update=[])
                        out.append(n)
                    ins.sync_info = mybir.SyncInfo(
                        on_wait=waits[-1:], on_update=list(si.on_update or [])
                    )
                out.append(ins)
            if changed:
                bb.instructions = out


def _build_nc():
    fp32 = mybir.dt.float32
    bf16 = mybir.dt.bfloat16
    SIG = mybir.ActivationFunctionType.Sigmoid

    nc = bass.Bass("TRN2", target_bir_lowering=False, debug=False)

    d_w1t = nc.dram_tensor("w1t", [LATENT, HIDDEN], bf16, kind="ExternalInput")
    d_wt = nc.dram_tensor("wt", [LATENT, BATCH], bf16, kind="ExternalInput")
    d_b1r = nc.dram_tensor("b1r", [1, HIDDEN], bf16, kind="ExternalInput")
    d_s = nc.dram_tensor(
        "sdense", [2, HB, 128, HRB, RBLK], bf16, kind="ExternalInput"
    )
    d_sc = nc.dram_tensor("screp", [128, RPC], fp32, kind="ExternalInput")
    d_rf = nc.dram_tensor("refrep", [128, RPC], fp32, kind="ExternalInput")
    d_out = nc.dram_tensor("out", [BATCH, RPC], fp32, kind="ExternalOutput")

    with TileContext(nc) as tc:
        with (
            tc.tile_pool(name="consts", bufs=1) as consts,
            tc.tile_pool(name="wstream", bufs=3) as wstream,
            tc.tile_pool(name="sstream", bufs=8) as sstream,
            tc.tile_pool(name="work", bufs=2) as work,
        ):
            # ---------------- small constant loads ----------------
            sb_wt = consts.tile([128, 2, BATCH], bf16)
            nc.sync.dma_start(
                out=sb_wt[:],
                in_=d_wt.ap().rearrange("(kc p) b -> p kc b", p=128),
            )
            sb_b1r = consts.tile([1, HIDDEN], bf16)
            nc.sync.dma_start(out=sb_b1r[:], in_=d_b1r.ap())
            sb_ones = consts.tile([1, BATCH], bf16)
            nc.gpsimd.memset(sb_ones[:], 1.0)

            # Collapse the per-lane DMA waits (most ISA structs hold 1 wait).
            _drain_barrier(tc, nc)

            # ---------------- stage A: h_T stays in SBUF ----------------
            ht_sb = consts.tile([128, HB, BATCH], bf16)
            w1t_ap = d_w1t.ap().rearrange("(kc p) h -> p kc h", p=128)
            with tc.tile_pool(name="psA", bufs=4, space="PSUM") as psA:
                for quad in range(HB // 4):
                    wq = wstream.tile([128, 2, 512], bf16, tag="wq")
                    nc.sync.dma_start(
                        out=wq[:], in_=w1t_ap[:, :, quad * 512 : (quad + 1) * 512]
                    )
                    ps = psA.tile([128, 512], fp32, tag="hps")
                    for i4 in range(4):
                        i = quad * 4 + i4
                        for k in range(2):
                            nc.tensor.matmul(
                                ps[:, i4 * 128 : (i4 + 1) * 128],
                                lhsT=wq[:, k, i4 * 128 : (i4 + 1) * 128],
                                rhs=sb_wt[:, k, :],
                                start=(k == 0),
                                stop=False,
                            )
                        # bias fold: h_T[h, b] += b1[h] * ones[b]
                        nc.tensor.matmul(
                            ps[:, i4 * 128 : (i4 + 1) * 128],
                            lhsT=sb_b1r[:, i * 128 : (i + 1) * 128],
                            rhs=sb_ones[:],
                            start=False,
                            stop=True,
                        )
                    nc.scalar.activation(
                        ht_sb[:, quad * 4 : (quad + 1) * 4, :],
                        ps[:].rearrange("p (q b) -> p q b", q=4),
                        SIG,
                        bias=0.0,
                        scale=1.0,
                    )

            # ---------------- stage B: dense S matmul, cc-major ----------------
            # Two passes over row halves; each pass streams all 64 hidden
            # chunks with one stationary load per chunk into 4 PSUM banks.
            with tc.tile_pool(name="psB", bufs=1, space="PSUM") as psB:
                for ph in range(2):
                    pss = [
                        psB.tile([128, RBLK], fp32, tag=f"ops{j}", name=f"ps{ph}_{j}")
                        for j in range(HRB)
                    ]
                    for cc in range(HB):
                        st = sstream.tile([128, HRB, RBLK], bf16, tag="s")
                        nc.sync.dma_start(out=st[:], in_=d_s.ap()[ph, cc])
                        for j in range(HRB):
                            nc.tensor.matmul(
                                pss[j][:],
                                lhsT=ht_sb[:, cc, :],
                                rhs=st[:, j, :],
                                start=(cc == 0),
                                stop=(cc == HB - 1),
                            )
                    if ph == 0:
                        # scale/ref land mid-stream on the SWDGE queue, off
                        # the latency-critical SP FIFO.
                        sb_sc = consts.tile([128, RPC], fp32)
                        nc.gpsimd.dma_start(out=sb_sc[:], in_=d_sc.ap())
                        sb_rf = consts.tile([128, RPC], fp32)
                        nc.gpsimd.dma_start(out=sb_rf[:], in_=d_rf.ap())
                    for j in range(HRB):
                        rb = ph * HRB + j
                        tmp = work.tile([128, RBLK], fp32, tag="tmp")
                        nc.vector.tensor_mul(
                            out=tmp[:],
                            in0=pss[j][:],
                            in1=sb_sc[:, rb * RBLK : (rb + 1) * RBLK],
                        )
                        ot = work.tile([128, RBLK], fp32, tag="ot")
                        nc.vector.tensor_add(
                            out=ot[:],
                            in0=tmp[:],
                            in1=sb_rf[:, rb * RBLK : (rb + 1) * RBLK],
                        )
                        nc.sync.dma_start(
                            out=d_out.ap()[:, rb * RBLK : (rb + 1) * RBLK], in_=ot[:]
                        )

    _split_multiwaits(nc)
    return nc


def _stage_inputs(w, W1, b1, edge_vals, rows, cols, scale, ref):
    """Pure-layout host staging: transposes, COO->dense canonicalization
    (duplicates summed, scipy-style), bf16 packing. No model arithmetic."""
    f32 = np.float32
    bf16 = ml_dtypes.bfloat16
    w = np.asarray(w, dtype=f32)
    W1 = np.asarray(W1, dtype=f32)
    b1 = np.asarray(b1, dtype=f32)
    edge_vals = np.asarray(edge_vals, dtype=f32)
    rows = np.asarray(rows, dtype=np.int64)
    cols = np.asarray(cols, dtype=np.int64)
    scale = np.asarray(scale, dtype=f32)
    ref = np.asarray(ref, dtype=f32)

    w1t = np.ascontiguousarray(W1.T.astype(bf16))         # [LATENT, HIDDEN]
    wt = np.ascontiguousarray(w.T.astype(bf16))           # [LATENT, BATCH]
    b1r = np.ascontiguousarray(b1.astype(bf16)[None, :])  # [1, HIDDEN]

    in_maps = []
    for k in range(NCORES):
        lo, hi = k * RPC, (k + 1) * RPC
        sel = (rows >= lo) & (rows < hi)
        r_k = rows[sel] - lo
        c_k = cols[sel]
        v_k = edge_vals[sel]

        # Dense per-core slice S[c, r], duplicate (c, r) entries summed.
        sdense = np.zeros((HIDDEN, RPC), dtype=f32)
        np.add.at(sdense, (c_k, r_k), v_k)
        # [2, HB, 128, HRB, RBLK]: pass-major, cc-major slabs, 4KB/partition
        sdense = sdense.reshape(HB, 128, 2, HRB, RBLK).transpose(2, 0, 1, 3, 4)
        sdense = np.ascontiguousarray(sdense.astype(bf16))

        screp = np.ascontiguousarray(np.broadcast_to(scale[lo:hi][None, :], (128, RPC)))
        refrep = np.ascontiguousarray(np.broadcast_to(ref[lo:hi][None, :], (128, RPC)))
        in_maps.append(
            {
                "w1t": w1t,
                "wt": wt,
                "b1r": b1r,
                "sdense": sdense,
                "screp": screp,
                "refrep": refrep,
            }
        )
    return in_maps


def kernel(w, W1, b1, edge_vals, rows, cols, scale, ref):
    in_maps = _stage_inputs(w, W1, b1, edge_vals, rows, cols, scale, ref)
    if "nc" not in _NC_CACHE:
        _NC_CACHE["nc"] = _build_nc()
    nc = _NC_CACHE["nc"]
    res = run_bass_kernel_spmd(nc, in_maps, core_ids=list(range(NCORES)))
    out = np.concatenate([r["out"] for r in res.results], axis=1)
    return out.astype(np.float32)


if __name__ == "__main__":
    rng = np.random.default_rng(0)
    nnz = OUT * 32
    ins = {
        "w": rng.standard_normal((BATCH, LATENT), dtype=np.float32),
        "W1": rng.standard_normal((HIDDEN, LATENT), dtype=np.float32),
        "b1": rng.standard_normal(HIDDEN, dtype=np.float32) * 0.01,
        "edge_vals": rng.standard_normal(nnz, dtype=np.float32),
        "rows": np.repeat(np.arange(OUT, dtype=np.int64), 32),
        "cols": rng.integers(0, HIDDEN, nnz).astype(np.int64),
        "scale": rng.random(OUT, dtype=np.float32) + 0.5,
        "ref": rng.standard_normal(OUT, dtype=np.float32),
    }
    out = kernel(**ins)
    print(out.shape, out.dtype)


# revision 16
# speedup vs baseline: 5.5583x; 1.0828x over previous
"""Trainium2 Bass kernel for nn_Decoder: dense MLP (sigmoid) + fixed-COO sparse matmul.

Computation:
    h = sigmoid(w @ W1.T + b1)                       # [B=128, H=8192]
    out_sp[b, r] = sum_e{rows[e]==r} edge_vals[e] * h[b, cols[e]]   # [B, OUT=32768]
    out = scale * out_sp + ref

Strategy (8 NeuronCores, SPMD, row-partitioned):
  - Core k owns output rows [4096k, 4096(k+1)).
  - The sparse matrix is canonicalized host-side (COO -> dense per-core
    slice, duplicates summed, bf16) and streamed slab-by-slab from HBM:
    measured dma_gather descriptor generation costs ~8 ns/edge on the Q7
    (1 ms total), while streaming the 64 MB/core dense slice runs at HBM
    rate (~160 us) and turns the sparse stage into dense matmuls with h
    resident in SBUF as the stationary operand (one LDWEIGHTS per hidden
    chunk, PE stays HAM-warm).
  - Dense stage replicated on every core; b1 folded in as a K=1 matmul so
    the sigmoid runs as one batched ACT per PSUM bank.
  - scale/ref fused into the PSUM evacuation on DVE; stage B runs in two
    row-half passes so the first half's evacuation overlaps the second
    half's S stream.
"""

import numpy as np
import ml_dtypes

import concourse.bass as bass
import concourse.mybir as mybir
from concourse.tile import TileContext
from concourse.bass_utils import run_bass_kernel_spmd

LATENT, HIDDEN, OUT, BATCH = 256, 8192, 32768, 128
NCORES = 8
RPC = OUT // NCORES          # rows per core = 4096
RBLK = 512                   # output rows per PSUM bank
NRB = RPC // RBLK            # 8 row blocks per core
HRB = NRB // 2               # row blocks per pass
HB = HIDDEN // 128           # 64 hidden chunks

_NC_CACHE = {}


def _drain_barrier(tc, nc):
    """strict_bb_all_engine_barrier, but with an SP drain instead of a nop
    (the nop ISA struct can't hold the many DMA-lane waits)."""
    from concourse.tile_rust import add_dep_helper

    curr_bb = nc.cur_bb
    assert curr_bb is not None
    prev = list(curr_bb.bb.instructions)
    b = nc.sync.drain(fusable=False)
    tc.barrier_instruction_and_bb = (b.ins, curr_bb)
    if (
        tc.no_sync_barrier_and_bb is not None
        and tc.no_sync_barrier_and_bb[1] == curr_bb
    ):
        tc.no_sync_barrier_and_bb = None
    for inst in prev:
        add_dep_helper(
            b.ins,
            inst,
            sync=bass.sync_unless_reorderable_target(inst, inst.is_executable()),
            reason="drain_barrier: backward edge",
        )


def _split_multiwaits(nc):
    """walrus codegen embeds at most ONE sync wait per ISA instruction and
    errors with "Too many sync wait commands" otherwise.  Split extra waits
    into single-wait NoOps on the same engine immediately before the
    instruction (engine streams keep program order through walrus)."""
    for f in nc.m.functions:
        for bb in f.blocks:
            out, changed = [], False
            for ins in bb.instructions:
                si = ins.sync_info
                waits = list(si.on_wait) if si and si.on_wait else []
                if len(waits) > 1:
                    changed = True
                    for wsub in waits[:-1]:
                        n = mybir.InstNoOp(name=f"I-{nc.next_id()}", ins=[], outs=[])
                        n.engine = ins.engine
                        n.sync_info = mybir.SyncInfo(on_wait=[wsub], on_update=[])
                        out.append(n)
                    ins.sync_info = mybir.SyncInfo(
                        on_wait=waits[-1:], on_update=list(si.on_update or [])
                    )
                out.append(ins)
            if changed:
                bb.instructions = out


def _build_nc():
    fp32 = mybir.dt.float32
    bf16 = mybir.dt.bfloat16
    SIG = mybir.ActivationFunctionType.Sigmoid

    nc = bass.Bass("TRN2", target_bir_lowering=False, debug=False)

    d_w1t = nc.dram_tensor("w1t", [LATENT, HIDDEN], bf16, kind="ExternalInput")
    d_wt = nc.dram_tensor("wt", [LATENT, BATCH], bf16, kind="ExternalInput")
    d_b1r = nc.dram_tensor("b1r", [1, HIDDEN], bf16, kind="ExternalInput")
    d_s = nc.dram_tensor(
        "sdense", [2, HB, 128, HRB, RBLK], bf16, kind="ExternalInput"
    )
    d_sc = nc.dram_tensor("screp", [128, RPC], fp32, kind="ExternalInput")
    d_rf = nc.dram_tensor("refrep", [128, RPC], fp32, kind="ExternalInput")
    d_out = nc.dram_tensor("out", [BATCH, RPC], fp32, kind="ExternalOutput")

    with TileContext(nc) as tc:
        with (
            tc.tile_pool(name="consts", bufs=1) as consts,
            tc.tile_pool(name="wstream", bufs=3) as wstream,
            # Deep prefetch: stage A runs ~50us on PE before the first
            # stage-B matmul; ~24MB of S buffer keeps the DMA engines
            # streaming continuously through it.
            tc.tile_pool(name="sstream", bufs=24) as sstream,
            tc.tile_pool(name="work", bufs=2) as work,
        ):
            # ---------------- small constant loads ----------------
            sb_wt = consts.tile([128, 2, BATCH], bf16)
            nc.sync.dma_start(
                out=sb_wt[:],
                in_=d_wt.ap().rearrange("(kc p) b -> p kc b", p=128),
            )
            sb_b1r = consts.tile([1, HIDDEN], bf16)
            nc.sync.dma_start(out=sb_b1r[:], in_=d_b1r.ap())
            sb_ones = consts.tile([1, BATCH], bf16)
            nc.gpsimd.memset(sb_ones[:], 1.0)

            # Collapse the per-lane DMA waits (most ISA structs hold 1 wait).
            _drain_barrier(tc, nc)

            # ---------------- stage A: h_T stays in SBUF ----------------
            ht_sb = consts.tile([128, HB, BATCH], bf16)
            w1t_ap = d_w1t.ap().rearrange("(kc p) h -> p kc h", p=128)
            with tc.tile_pool(name="psA", bufs=4, space="PSUM") as psA:
                for quad in range(HB // 4):
                    wq = wstream.tile([128, 2, 512], bf16, tag="wq")
                    nc.sync.dma_start(
                        out=wq[:], in_=w1t_ap[:, :, quad * 512 : (quad + 1) * 512]
                    )
                    ps = psA.tile([128, 512], fp32, tag="hps")
                    for i4 in range(4):
                        i = quad * 4 + i4
                        for k in range(2):
                            nc.tensor.matmul(
                                ps[:, i4 * 128 : (i4 + 1) * 128],
                                lhsT=wq[:, k, i4 * 128 : (i4 + 1) * 128],
                                rhs=sb_wt[:, k, :],
                                start=(k == 0),
                                stop=False,
                            )
                        # bias fold: h_T[h, b] += b1[h] * ones[b]
                        nc.tensor.matmul(
                            ps[:, i4 * 128 : (i4 + 1) * 128],
                            lhsT=sb_b1r[:, i * 128 : (i + 1) * 128],
                            rhs=sb_ones[:],
                            start=False,
                            stop=True,
                        )
                    nc.scalar.activation(
                        ht_sb[:, quad * 4 : (quad + 1) * 4, :],
                        ps[:].rearrange("p (q b) -> p q b", q=4),
                        SIG,
                        bias=0.0,
                        scale=1.0,
                    )

            # ---------------- stage B: dense S matmul, cc-major ----------------
            # Two passes over row halves; each pass streams all 64 hidden
            # chunks with one stationary load per chunk into 4 PSUM banks.
            with tc.tile_pool(name="psB", bufs=1, space="PSUM") as psB:
                for ph in range(2):
                    pss = [
                        psB.tile([128, RBLK], fp32, tag=f"ops{j}", name=f"ps{ph}_{j}")
                        for j in range(HRB)
                    ]
                    for cc in range(HB):
                        st = sstream.tile([128, HRB, RBLK], bf16, tag="s")
                        nc.sync.dma_start(out=st[:], in_=d_s.ap()[ph, cc])
                        for j in range(HRB):
                            nc.tensor.matmul(
                                pss[j][:],
                                lhsT=ht_sb[:, cc, :],
                                rhs=st[:, j, :],
                                start=(cc == 0),
                                stop=(cc == HB - 1),
                            )
                    if ph == 0:
                        # scale/ref land mid-stream on the SWDGE queue, off
                        # the latency-critical SP FIFO.
                        sb_sc = consts.tile([128, RPC], fp32)
                        nc.gpsimd.dma_start(out=sb_sc[:], in_=d_sc.ap())
                        sb_rf = consts.tile([128, RPC], fp32)
                        nc.gpsimd.dma_start(out=sb_rf[:], in_=d_rf.ap())
                    for j in range(HRB):
                        rb = ph * HRB + j
                        tmp = work.tile([128, RBLK], fp32, tag="tmp")
                        nc.vector.tensor_mul(
                            out=tmp[:],
                            in0=pss[j][:],
                            in1=sb_sc[:, rb * RBLK : (rb + 1) * RBLK],
                        )
                        ot = work.tile([128, RBLK], fp32, tag="ot")
                        nc.vector.tensor_add(
                            out=ot[:],
                            in0=tmp[:],
                            in1=sb_rf[:, rb * RBLK : (rb + 1) * RBLK],
                        )
                        nc.sync.dma_start(
                            out=d_out.ap()[:, rb * RBLK : (rb + 1) * RBLK], in_=ot[:]
                        )

    _split_multiwaits(nc)
    return nc


def _stage_inputs(w, W1, b1, edge_vals, rows, cols, scale, ref):
    """Pure-layout host staging: transposes, COO->dense canonicalization
    (duplicates summed, scipy-style), bf16 packing. No model arithmetic."""
    f32 = np.float32
    bf16 = ml_dtypes.bfloat16
    w = np.asarray(w, dtype=f32)
    W1 = np.asarray(W1, dtype=f32)
    b1 = np.asarray(b1, dtype=f32)
    edge_vals = np.asarray(edge_vals, dtype=f32)
    rows = np.asarray(rows, dtype=np.int64)
    cols = np.asarray(cols, dtype=np.int64)
    scale = np.asarray(scale, dtype=f32)
    ref = np.asarray(ref, dtype=f32)

    w1t = np.ascontiguousarray(W1.T.astype(bf16))         # [LATENT, HIDDEN]
    wt = np.ascontiguousarray(w.T.astype(bf16))           # [LATENT, BATCH]
    b1r = np.ascontiguousarray(b1.astype(bf16)[None, :])  # [1, HIDDEN]

    in_maps = []
    for k in range(NCORES):
        lo, hi = k * RPC, (k + 1) * RPC
        sel = (rows >= lo) & (rows < hi)
        r_k = rows[sel] - lo
        c_k = cols[sel]
        v_k = edge_vals[sel]

        # Dense per-core slice S[c, r], duplicate (c, r) entries summed.
        sdense = np.zeros((HIDDEN, RPC), dtype=f32)
        np.add.at(sdense, (c_k, r_k), v_k)
        # [2, HB, 128, HRB, RBLK]: pass-major, cc-major slabs, 4KB/partition
        sdense = sdense.reshape(HB, 128, 2, HRB, RBLK).transpose(2, 0, 1, 3, 4)
        sdense = np.ascontiguousarray(sdense.astype(bf16))

        screp = np.ascontiguousarray(np.broadcast_to(scale[lo:hi][None, :], (128, RPC)))
        refrep = np.ascontiguousarray(np.broadcast_to(ref[lo:hi][None, :], (128, RPC)))
        in_maps.append(
            {
                "w1t": w1t,
                "wt": wt,
                "b1r": b1r,
                "sdense": sdense,
                "screp": screp,
                "refrep": refrep,
            }
        )
    return in_maps


def kernel(w, W1, b1, edge_vals, rows, cols, scale, ref):
    in_maps = _stage_inputs(w, W1, b1, edge_vals, rows, cols, scale, ref)
    if "nc" not in _NC_CACHE:
        _NC_CACHE["nc"] = _build_nc()
    nc = _NC_CACHE["nc"]
    res = run_bass_kernel_spmd(nc, in_maps, core_ids=list(range(NCORES)))
    out = np.concatenate([r["out"] for r in res.results], axis=1)
    return out.astype(np.float32)


if __name__ == "__main__":
    rng = np.random.default_rng(0)
    nnz = OUT * 32
    ins = {
        "w": rng.standard_normal((BATCH, LATENT), dtype=np.float32),
        "W1": rng.standard_normal((HIDDEN, LATENT), dtype=np.float32),
        "b1": rng.standard_normal(HIDDEN, dtype=np.float32) * 0.01,
        "edge_vals": rng.standard_normal(nnz, dtype=np.float32),
        "rows": np.repeat(np.arange(OUT, dtype=np.int64), 32),
        "cols": rng.integers(0, HIDDEN, nnz).astype(np.int64),
        "scale": rng.random(OUT, dtype=np.float32) + 0.5,
        "ref": rng.standard_normal(OUT, dtype=np.float32),
    }
    out = kernel(**ins)
    print(out.shape, out.dtype)
